# revision 70
# baseline (speedup 1.0000x reference)
"""Fused KAN-linear multi-head-attention kernel for 8 Trainium2 NeuronCores.

Sharding: data-parallel over (batch=4) x (sequence halves=2) -> 8 cores.
Each core computes Q/K/V KAN projections for its 512 tokens, pairs exchange
rope'd K and token-major V via an in-pair AllGather, then each core runs
attention for its 512 queries over the full 1024-key sequence and the output
projection for its tokens.

The KAN spline bases are evaluated in closed form: on the uniform grid all 8
cubic B-spline bases are translates of the cardinal cubic B-spline,
  B_c(t) = (1/6)[u^3 - 4 relu(u-1)^3 + 6 relu(u-2)^3 - 4 relu(u-3)^3],
  u = clamp(t - c, 0, 4),  t = 2.5 x + 5.5,
computed from clamped cubes of relu(t-k) with exact zero tails.  The 1/6 and
the per-(out,in) scaler are folded into the spline weights host-side.

The spline contraction (6144 deep, 89% of the KAN FLOPs) runs in fp8e4m3
with DoubleRow perf mode: weights and bases are quantized to fp8 (weights
pre-scaled x512 to sit in e4m3's normal range), consecutive k-tile pairs are
packed into [128, 2, N] APs, and each matmul contracts 256 rows per pass.
The base (silu) matmul runs in bf16 with weights also pre-scaled x512 so
both accumulate into the same PSUM; the 1/512 is folded into the
PSUM-evacuation copy for free.  Q/K/V, the attention operands and the output
projection all run in bf16 (fp32r streams at half rate on the PE, bf16 at
full rate; this also halves the K/V pair-exchange).  The cube chain for the
spline bases is split across Scalar/Vector/GpSimd so basis production keeps
up with the fp8 matmul stream, and the softmax reciprocals are batched into
one 12-partition DVE op instead of twelve 1-partition ones.
"""

import numpy as np
import ml_dtypes

import concourse.bass as bass
import concourse.tile as tile
from concourse import bacc, mybir
from concourse import bass_utils

F32 = mybir.dt.float32
F32R = mybir.dt.float32r
BF16 = mybir.dt.bfloat16
FP8 = mybir.dt.float8e4
ALU = mybir.AluOpType
ACTF = mybir.ActivationFunctionType
DR = mybir.MatmulPerfMode.DoubleRow

B, S, H = 4, 1024, 768
HEADS, D = 12, 64
SH = 512                  # tokens per core
NDK = 24                  # spline double-k-tiles (48 k-tiles paired)
N_CORES = 8
ROPE_THETA = 10000.0
WSCALE = 512.0            # host pre-scale on all KAN weights
INV_WSCALE = 1.0 / WSCALE

K_ELEMS = H * SH                # rope'd K, feature-major [768, 512]
VT_COLS = HEADS * 65            # token-major V + per-head ones column
V_ELEMS = SH * VT_COLS          # [512, 780]
KV_ELEMS = K_ELEMS + V_ELEMS

_CACHE = {}
DEBUG = False


def _trunc_f32r(a):
    a = np.ascontiguousarray(a, np.float32)
    return (a.view(np.uint32) & np.uint32(0xFFFFFC00)).view(np.float32)


def _host_prep(inputs):
    """Rearrange weights and build per-core input maps."""
    f32 = np.float32
    bf16 = ml_dtypes.bfloat16
    fp8 = ml_dtypes.float8_e4m3
    x = np.asarray(inputs["x"], f32)

    shared = {}
    for lin in ("q", "k", "v"):
        sw = np.asarray(inputs[f"{lin}_spline_w"], f32)      # (o, i, c)
        sc = np.asarray(inputs[f"{lin}_scaler"], f32)
        ws = sw * sc[..., None] * np.float32(WSCALE / 6.0)
        # rows ordered (i_block, c, i_within); cols = o
        t2 = ws.transpose(1, 2, 0).reshape(6, 128, 8, H)      # (ib, di, c, o)
        w48 = t2.transpose(0, 2, 1, 3).reshape(48, 128, H)    # kt=(ib*8+c)
        if lin == "v":
            # V error passes linearly to the output -> bf16 spline weights
            # (moving operand of the token-major matmul, fp8 bases stay
            # the stationary operand)
            shared["w2v"] = w48.reshape(48 * 128, H).astype(bf16)
        else:
            # DoubleRow pairs: [dk, di, sub, o] flattened to [3072, 1536]
            wdr = (w48.reshape(NDK, 2, 128, H).transpose(0, 2, 1, 3)
                   .reshape(NDK * 128, 2 * H))
            shared[f"w2{lin}"] = np.clip(wdr, -240.0, 240.0).astype(fp8)
        shared[f"bw{lin}"] = (
            np.asarray(inputs[f"{lin}_base_w"], f32).T * np.float32(WSCALE)
        ).astype(bf16)                                        # (i, o)
    shared["owT"] = np.asarray(inputs["out_w"], f32).T.astype(bf16)
    shared["ob"] = _trunc_f32r(
        np.asarray(inputs["out_b"], f32).reshape(1, H))

    # rotate-half matrix as lhsT: (lhsT.T @ u) = rot_half(u), 2 heads/tile
    rotT = np.zeros((128, 128), f32)
    for blk in (0, 64):
        for j in range(32):
            rotT[blk + j + 32, blk + j] = -1.0
            rotT[blk + j, blk + j + 32] = 1.0
    shared["rotT"] = _trunc_f32r(rotT)

    # rope tables per sequence half: [64 d, 512 pos] tiled to 128 partitions
    inv_freq = (1.0 / (ROPE_THETA ** (np.arange(0, D, 2, dtype=f32) / D))).astype(f32)
    cos_h, sin_h = [], []
    for half in range(2):
        pos = (half * SH + np.arange(SH)).astype(f32)
        freqs = np.outer(pos, inv_freq).astype(f32)           # (512, 32)
        emb = np.concatenate([freqs, freqs], axis=1)          # (512, 64)
        cos_h.append(np.tile(np.cos(emb).astype(f32).T, (2, 1)))  # (128, 512)
        sin_h.append(np.tile(np.sin(emb).astype(f32).T, (2, 1)))

    in_maps = []
    for c in range(N_CORES):
        b, half = c // 2, c % 2
        m = dict(shared)
        xT = x[b, half * SH:(half + 1) * SH, :].T          # (768, 512)
        # super-blocks: row j*128+p holds features (2j)*128+p | (2j+1)*128+p
        # side by side so the basis pipeline runs on [128, 1024] tiles
        m["xT"] = np.ascontiguousarray(
            xT.reshape(3, 2, 128, SH).transpose(0, 2, 1, 3).reshape(384, 2 * SH))
        m["cosT"] = cos_h[half]
        m["sinT"] = sin_h[half]
        in_maps.append(m)
    return in_maps


def _build_program():
    nc = bacc.Bacc("TRN2", target_bir_lowering=False, debug=False,
                   num_devices=N_CORES)

    d_xT = nc.dram_tensor("xT", [384, 2 * SH], F32, kind="ExternalInput").ap()
    d_cos = nc.dram_tensor("cosT", [128, SH], F32, kind="ExternalInput").ap()
    d_sin = nc.dram_tensor("sinT", [128, SH], F32, kind="ExternalInput").ap()
    d_rot = nc.dram_tensor("rotT", [128, 128], F32R, kind="ExternalInput").ap()
    d_w2 = {lin: nc.dram_tensor(f"w2{lin}", [NDK * 128, 2 * H], FP8,
                                kind="ExternalInput").ap()
            for lin in ("q", "k")}
    d_w2["v"] = nc.dram_tensor("w2v", [48 * 128, H], BF16,
                               kind="ExternalInput").ap()
    d_bw = {lin: nc.dram_tensor(f"bw{lin}", [H, H], BF16,
                                kind="ExternalInput").ap()
            for lin in ("q", "k", "v")}
    d_ow = nc.dram_tensor("owT", [H, H], BF16, kind="ExternalInput").ap()
    d_ob = nc.dram_tensor("ob", [1, H], F32R, kind="ExternalInput").ap()
    d_out = nc.dram_tensor("out", [SH, H], F32, kind="ExternalOutput").ap()
    dbg = {}
    if DEBUG:
        for nm, shp, dt in [("dbg_bases", [128, 4 * SH], FP8),
                            ("dbg_silu", [128, 2 * SH], BF16),
                            ("dbg_kraw", [128, SH], F32),
                            ("dbg_kf", [128, SH], BF16),
                            ("dbg_vt", [128, VT_COLS], BF16),
                            ("dbg_qf", [128, SH], BF16),
                            ("dbg_ka", [128, SH], BF16),
                            ("dbg_va", [128, VT_COLS], BF16),
                            ("dbg_pt", [128, SH], BF16),
                            ("dbg_at", [128, SH], F32),
                            ("dbg_af", [128, SH], BF16)]:
            dbg[nm] = nc.dram_tensor(nm, shp, dt, kind="ExternalOutput").ap()

    with tile.TileContext(nc) as tc:
        _emit(nc, tc, d_xT, d_cos, d_sin, d_rot, d_w2, d_bw, d_ow, d_ob, d_out,
              dbg)
    nc.compile()
    return nc


def _emit(nc, tc, d_xT, d_cos, d_sin, d_rot, d_w2, d_bw, d_ow, d_ob, d_out,
          dbg={}):
    from contextlib import ExitStack

    ctx = ExitStack()
    with ctx:
        const = ctx.enter_context(tc.tile_pool(name="const", bufs=1))
        wpool = ctx.enter_context(tc.tile_pool(name="wpool", bufs=4))
        bwpool = ctx.enter_context(tc.tile_pool(name="bwpool", bufs=2))
        qf_pool = ctx.enter_context(tc.tile_pool(name="qfp", bufs=6))
        kf_pool = ctx.enter_context(tc.tile_pool(name="kfp", bufs=2))
        vt_pool = ctx.enter_context(tc.tile_pool(name="vtp", bufs=2))
        dram = ctx.enter_context(tc.tile_pool(name="dram", bufs=1, space="DRAM"))

        cosT = const.tile([128, SH], F32, tag="cos")
        nc.sync.dma_start(cosT[:], d_cos[:])
        sinT = const.tile([128, SH], F32, tag="sin")
        nc.sync.dma_start(sinT[:], d_sin[:])
        rotT = const.tile([128, 128], F32R, tag="rot")
        nc.sync.dma_start(rotT[:], d_rot[:])
        ob_sb = const.tile([1, H], F32, tag="ob")
        nc.sync.dma_start(ob_sb[:], d_ob.bitcast(F32)[:])
        bias_bc = const.tile([128, H], F32, tag="biasbc")
        nc.gpsimd.partition_broadcast(bias_bc[:], ob_sb[:])

        # single combined exchange buffer: each pair-AllGather costs ~20us of
        # FIXED sync overhead regardless of bytes and holds the gpsimd queue,
        # so one collective after K+V beats two queued ones
        kv_in = dram.tile([KV_ELEMS], BF16, tag="kvin")
        kv_out = dram.tile([2 * KV_ELEMS], BF16, tag="kvout")

        neg_k = []
        for k in range(11):
            bk = const.tile([128, 1], F32, tag=f"bk{k}")
            nc.vector.memset(bk[:], float(-k))
            neg_k.append(bk)
        ones_p = const.tile([128, 1], F32, tag="onesp")
        nc.vector.memset(ones_p[:], 1.0)
        rbias = {}
        for v in (64.0, 108.0, 48.0, 4.0):
            bv = const.tile([128, 1], F32, tag=f"rb{int(v)}")
            nc.vector.memset(bv[:], v)
            rbias[v] = bv

        silu = []
        bases = {}                # (ib, m) -> [128, 2*SH] fp8 pair tile

        # ---------------- bases + silu ----------------
        bases_ctx = ExitStack()
        bases_pool = bases_ctx.enter_context(
            tc.tile_pool(name="basesp", bufs=12))
        silu_pool = bases_ctx.enter_context(tc.tile_pool(name="silup", bufs=3))
        rs_pool = bases_ctx.enter_context(tc.tile_pool(name="rsp", bufs=2))
        scr = bases_ctx.enter_context(tc.tile_pool(name="scr", bufs=2))
        scr_kp = bases_ctx.enter_context(tc.tile_pool(name="scrkp", bufs=5))
        scr_b = bases_ctx.enter_context(tc.tile_pool(name="scrb", bufs=5))
        xp = bases_ctx.enter_context(tc.tile_pool(name="xp", bufs=2))

        # One pass per super-block j covers real feature blocks (2j, 2j+1)
        # side by side in [128, 1024] tiles: halves the elementwise op count
        # so the per-op engine overhead is paid half as often.
        W = 2 * SH
        for j in range(3):
            xt = xp.tile([128, W], F32, tag="x")
            nc.sync.dma_start(xt[:], d_xT[j * 128:(j + 1) * 128, :])
            sl = silu_pool.tile([128, W], BF16, tag="silu")
            nc.scalar.activation(sl[:], xt[:], ACTF.Silu)
            silu.append(sl)

            tt = scr.tile([128, W], F32, tag="t")
            nc.vector.tensor_scalar(tt[:], xt[:], 2.5, 5.5, ALU.mult, ALU.add)

            # chain intermediates in bf16. DVE is ALU-bound (not stream-
            # bound), so minimize op cost: plain cubes via tensor_tensor,
            # each clamped term as ONE tensor_scalar (scale folded into the
            # clamp op), and plain tensor_tensor adds to combine:
            #   B~ = min(C,64) + max(-4C',-108) + min(6C'',48) + max(-4C''',-4)
            M0 = {}
            M1 = {}
            M2 = {}
            M3 = {}
            CB = {}
            for k in range(11):
                R = scr.tile([128, W], BF16, tag="R")
                nc.scalar.activation(R[:], tt[:], ACTF.Relu, bias=neg_k[k][:])
                Sq = scr.tile([128, W], BF16, tag="Sq")
                nc.scalar.activation(Sq[:], R[:], ACTF.Square)
                cb = scr_kp.tile([128, W], BF16, tag="Kp")
                nc.vector.tensor_mul(cb[:], Sq[:], R[:])   # relu(t-k)^3
                CB[k] = cb
                if k <= 5:
                    m0 = scr_b.tile([128, W], BF16, tag="m0")
                    nc.vector.tensor_scalar(m0[:], cb[:], 1.0, 64.0,
                                            ALU.mult, ALU.min)
                    M0[k] = m0
                if 1 <= k <= 6:
                    m1 = scr_b.tile([128, W], BF16, tag="m1")
                    nc.vector.tensor_scalar(m1[:], cb[:], -4.0, -108.0,
                                            ALU.mult, ALU.max)
                    M1[k] = m1
                if 2 <= k <= 7:
                    m2 = scr_b.tile([128, W], BF16, tag="m2")
                    nc.vector.tensor_scalar(m2[:], cb[:], 6.0, 48.0,
                                            ALU.mult, ALU.min)
                    M2[k] = m2
                if 3 <= k <= 8:
                    m3 = scr_b.tile([128, W], BF16, tag="m3")
                    nc.vector.tensor_scalar(m3[:], cb[:], -4.0, -4.0,
                                            ALU.mult, ALU.max)
                    M3[k] = m3
                if k >= 3:
                    c = k - 3
                    if c % 2 == 0:
                        bp = bases_pool.tile([128, 2 * W], FP8, tag="bases")
                        bases[(j, c // 2)] = bp
                    bp4 = bases[(j, c // 2)].rearrange(
                        "p (i s n) -> p i s n", i=2, s=2)
                    acc = scr.tile([128, W], BF16, tag="acc")
                    if c < 6:
                        nc.vector.tensor_add(acc[:], M0[c][:], M1[c + 1][:])
                        nc.vector.tensor_add(acc[:], acc[:], M2[c + 2][:])
                        # final add -> fp8 quarter of the 4D tile
                        # layout [128, (ib2, s, n)]: per real ib a
                        # [128, 2, 512] DoubleRow pair AP, ib2 axis strided.
                        nc.vector.tensor_add(
                            bp4[:, :, c % 2, :],
                            acc.rearrange("p (i n) -> p i n", i=2),
                            M3[c + 3].rearrange("p (i n) -> p i n", i=2))
                    else:
                        # ACT-relu form for the last two bases rebalances the
                        # clamp work from the (saturated) DVE onto Scalar:
                        # B~ = (r1 + r3) - (r0 + r2), r_j = relu(b_j - a_j*C)
                        # with the shift constants cancelling exactly.
                        r0 = scr_b.tile([128, W], BF16, tag="m0")
                        nc.scalar.activation(r0[:], CB[c][:], ACTF.Relu,
                                             scale=-1.0, bias=rbias[64.0][:])
                        r1 = scr_b.tile([128, W], BF16, tag="m1")
                        nc.scalar.activation(r1[:], CB[c + 1][:], ACTF.Relu,
                                             scale=-4.0, bias=rbias[108.0][:])
                        r2 = scr_b.tile([128, W], BF16, tag="m2")
                        nc.scalar.activation(r2[:], CB[c + 2][:], ACTF.Relu,
                                             scale=-6.0, bias=rbias[48.0][:])
                        r3 = scr_b.tile([128, W], BF16, tag="m3")
                        nc.scalar.activation(r3[:], CB[c + 3][:], ACTF.Relu,
                                             scale=-4.0, bias=rbias[4.0][:])
                        nc.vector.tensor_add(acc[:], r1[:], r3[:])
                        a2 = scr.tile([128, W], BF16, tag="acc2")
                        nc.vector.tensor_add(a2[:], r0[:], r2[:])
                        nc.vector.tensor_sub(
                            bp4[:, :, c % 2, :],
                            acc.rearrange("p (i n) -> p i n", i=2),
                            a2.rearrange("p (i n) -> p i n", i=2))
            del M0, M1, M2, M3, CB
            if dbg and j == 0:
                nc.sync.dma_start(dbg["dbg_bases"], bases[(0, 3)][:])
                nc.sync.dma_start(dbg["dbg_silu"], silu[0][:])

        # ---------------- KAN matmul phases ----------------
        def spline_w(lin, dk):
            wt = wpool.tile([128, 2 * H], FP8, tag="w8")
            nc.sync.dma_start(wt[:], d_w2[lin][dk * 128:(dk + 1) * 128, :])
            return wt.rearrange("p (s o) -> p s o", s=2)

        def base_w(lin, ib):
            bwt = bwpool.tile([128, H], BF16, tag="wb")
            nc.sync.dma_start(bwt[:], d_bw[lin][ib * 128:(ib + 1) * 128, :])
            return bwt

        def feature_major_phase(lin, out_pool, out_tag, bounce_base):
            """Q/K: out[o, n] with rope; optionally DMA to exchange buffer."""
            outs = []
            with tc.tile_pool(name=f"ps_{lin}", bufs=6, space="PSUM") as psp, \
                 tc.tile_pool(name=f"aux_{lin}", bufs=2, space="PSUM") as aux:
                ps = [psp.tile([128, SH], F32, tag="kan", name=f"kan{i}")
                      for i in range(6)]
                # base (silu) matmuls first: they only need silu, so the PE
                # has work while the spline bases are still being produced
                for ib in range(6):
                    bwt = base_w(lin, ib)
                    sl = silu[ib // 2][:, (ib % 2) * SH:(ib % 2 + 1) * SH]
                    for ot in range(6):
                        nc.tensor.matmul(ps[ot][:],
                                         bwt[:, ot * 128:(ot + 1) * 128],
                                         sl,
                                         start=(ib == 0), stop=False)
                for dk in range(NDK):
                    ib, m = dk // 4, dk % 4
                    w3 = spline_w(lin, dk)
                    b3 = bases[(ib // 2, m)].rearrange(
                        "p (i s n) -> p i s n", i=2, s=2)[:, ib % 2, :, :]
                    for ot in range(6):
                        nc.tensor.matmul(ps[ot][:],
                                         w3[:, :, ot * 128:(ot + 1) * 128],
                                         b3[:, :, :],
                                         start=False, stop=(dk == NDK - 1),
                                         perf_mode=DR)
                for ot in range(6):
                    raw = rs_pool.tile([128, SH], F32R, tag="raw")
                    nc.scalar.activation(raw[:], ps[ot][:], ACTF.Copy,
                                         scale=INV_WSCALE)
                    if dbg and lin == "k" and ot == 0:
                        nc.sync.dma_start(dbg["dbg_kraw"], raw.bitcast(F32)[:])
                    rp = aux.tile([128, SH], F32, tag="rope")
                    nc.tensor.matmul(rp[:], rotT[:], raw[:],
                                     start=True, stop=True)
                    t1 = rs_pool.tile([128, SH], F32, tag="rt1")
                    nc.vector.tensor_mul(t1[:], raw.bitcast(F32)[:], cosT[:])
                    t2 = rs_pool.tile([128, SH], F32, tag="rt2")
                    nc.vector.tensor_mul(t2[:], rp[:], sinT[:])
                    out = out_pool.tile([128, SH], BF16, tag=out_tag)
                    nc.vector.tensor_add(out[:], t1[:], t2[:])
                    outs.append(out)
                    if dbg and ot == 0:
                        nc.sync.dma_start(
                            dbg["dbg_kf" if lin == "k" else "dbg_qf"], out[:])
                    if bounce_base is not None:
                        off = bounce_base + ot * 128 * SH
                        dst = kv_in[off:off + 128 * SH].rearrange(
                            "(p f) -> p f", f=SH)
                        nc.sync.dma_start(dst, out[:])
            return outs

        def token_major_v_phase():
            """V: out[n, heads*65] with ones columns, to exchange buffer."""
            with tc.tile_pool(name="ps_v", bufs=8, space="PSUM") as pv:
                ps = [pv.tile([128, 384], F32, tag="vps", name=f"vps{i}")
                      for i in range(8)]
                for ib in range(6):
                    bwt = base_w("v", ib)
                    sl = silu[ib // 2]
                    for nb in range(4):
                        for hf in range(2):
                            nc.tensor.matmul(
                                ps[nb * 2 + hf][:],
                                sl[:, (ib % 2) * SH + nb * 128:
                                   (ib % 2) * SH + (nb + 1) * 128],
                                bwt[:, hf * 384:(hf + 1) * 384],
                                start=(ib == 0), stop=False)
                for kt in range(48):
                    ib, c = kt // 8, kt % 8
                    wt = wpool.tile([128, H], BF16, tag="wv")
                    nc.sync.dma_start(wt[:],
                                      d_w2["v"][kt * 128:(kt + 1) * 128, :])
                    bp4 = bases[(ib // 2, c // 2)].rearrange(
                        "p (i s n) -> p i s n", i=2, s=2)
                    for nb in range(4):
                        for hf in range(2):
                            nc.tensor.matmul(
                                ps[nb * 2 + hf][:],
                                bp4[:, ib % 2, c % 2,
                                    nb * 128:(nb + 1) * 128],
                                wt[:, hf * 384:(hf + 1) * 384],
                                start=False, stop=(kt == 47))
                for nb in range(4):
                    vt = vt_pool.tile([128, VT_COLS], BF16, tag="vt")
                    vt3 = vt.rearrange("p (h e) -> p h e", e=65)
                    nc.vector.tensor_copy(
                        vt3[:, :, 64:65],
                        ones_p[:].unsqueeze(2).to_broadcast([128, HEADS, 1]))
                    for hf in range(2):
                        src = ps[nb * 2 + hf].rearrange("p (h d) -> p h d", d=D)
                        nc.vector.tensor_scalar_mul(
                            vt3[:, hf * 6:(hf + 1) * 6, 0:D], src[:],
                            INV_WSCALE)
                    off = K_ELEMS + nb * 128 * VT_COLS
                    dst = kv_in[off:off + 128 * VT_COLS].rearrange(
                        "(p f) -> p f", f=VT_COLS)
                    nc.sync.dma_start(dst, vt[:])
                    if dbg and nb == 0:
                        nc.sync.dma_start(dbg["dbg_vt"], vt[:])

        # V first: its per-basis matmul volume best matches the basis
        # production rate, so the PE stays dense while bases trickle in.
        # One combined KV exchange after K: hides under the Q phase.
        groups = [[i, i + 1] for i in range(0, N_CORES, 2)]
        token_major_v_phase()
        feature_major_phase("k", kf_pool, "kf", bounce_base=0)
        nc.gpsimd.collective_compute(
            "AllGather", ALU.bypass, replica_groups=groups,
            ins=[kv_in.opt()], outs=[kv_out.opt()])
        qf = feature_major_phase("q", qf_pool, "qf", bounce_base=None)

        # close bases/scratch pools before attention working set opens
        bases_ctx.close()

        # ---------------- attention ----------------
        attn_ctx = ExitStack()
        ka_pool = attn_ctx.enter_context(tc.tile_pool(name="kap", bufs=12))
        va_pool = attn_ctx.enter_context(tc.tile_pool(name="vap", bufs=8))
        pb_pool = attn_ctx.enter_context(tc.tile_pool(name="pbp", bufs=10))
        an_pool = attn_ctx.enter_context(tc.tile_pool(name="anp", bufs=12))
        af_pool = attn_ctx.enter_context(tc.tile_pool(name="afp", bufs=6))
        sm_pool = attn_ctx.enter_context(tc.tile_pool(name="smp", bufs=4))
        out_pool = attn_ctx.enter_context(tc.tile_pool(name="outp", bufs=4))

        k_all = []
        for hf in range(2):
            for ot in range(6):
                t = ka_pool.tile([128, SH], BF16, tag="ka")
                off = hf * KV_ELEMS + ot * 128 * SH
                nc.sync.dma_start(
                    t[:],
                    kv_out[off:off + 128 * SH].rearrange("(p f) -> p f", f=SH))
                if dbg and hf == 1 and ot == 0:
                    nc.sync.dma_start(dbg["dbg_ka"], t[:])
                k_all.append(t)
        v_all = []
        for hf in range(2):
            for nb in range(4):
                t = va_pool.tile([128, VT_COLS], BF16, tag="va")
                off = hf * KV_ELEMS + K_ELEMS + nb * 128 * VT_COLS
                nc.sync.dma_start(
                    t[:],
                    kv_out[off:off + 128 * VT_COLS].rearrange(
                        "(p f) -> p f", f=VT_COLS))
                if dbg and hf == 1 and nb == 0:
                    nc.sync.dma_start(dbg["dbg_va"], t[:])
                v_all.append(t)

        af = [af_pool.tile([128, SH], BF16, tag="af", name=f"af{i}")
              for i in range(6)]

        # numerator + denominator evacuate to SBUF right after each head's
        # accumulation, so the at_ps bank frees in ~0.7us and the reciprocal
        # chain (reciprocal -> broadcast -> normalize) trails off-PSUM.
        # Everything stays at partition base 0: non-zero partition bases in
        # the copy/broadcast path silently misbehave on hardware.
        att_n = [an_pool.tile([D, SH], F32, tag="attn", name=f"attn{h}")
                 for h in range(HEADS)]

        pending = None
        with tc.tile_pool(name="ps_sc", bufs=3, space="PSUM") as scp, \
             tc.tile_pool(name="ps_at", bufs=2, space="PSUM") as atp:
            for h in range(HEADS):
                tf, r0 = h // 2, (h % 2) * D
                at_ps = atp.tile([128, SH], F32, tag="at")
                pts = []
                # score matmuls fill a 2-bank psum pair-wise; one exp call
                # covers both banks; the attn@V accumulations follow so exp
                # latency hides under the score stream
                for pr in range(4):
                    sc_ps = scp.tile([128, 2 * SH], F32, tag="sc")
                    for i in range(2):
                        kb = pr * 2 + i
                        hf, blk = kb // 4, kb % 4
                        nc.tensor.matmul(
                            sc_ps[:, i * SH:(i + 1) * SH],
                            k_all[hf * 6 + tf][r0:r0 + D,
                                               blk * 128:(blk + 1) * 128],
                            qf[tf][r0:r0 + D, :],
                            start=True, stop=True)
                    pt = pb_pool.tile([128, 2 * SH], BF16, tag="pt")
                    nc.scalar.activation(pt[:], sc_ps[:], ACTF.Exp,
                                         scale=0.125)
                    if dbg and h == 0 and pr == 0:
                        nc.sync.dma_start(dbg["dbg_pt"], pt[:, 0:SH])
                    pts.append(pt)
                for kb in range(8):
                    nc.tensor.matmul(at_ps[0:65, :],
                                     v_all[kb][:, h * 65:h * 65 + 65],
                                     pts[kb // 2][:, (kb % 2) * SH:
                                                  (kb % 2 + 1) * SH],
                                     start=(kb == 0), stop=(kb == 7))
                if dbg and h == 0:
                    dtmp = sm_pool.tile([128, SH], F32, tag="dbgat")
                    nc.vector.memset(dtmp[:], 0.0)
                    nc.vector.tensor_copy(dtmp[0:65, :], at_ps[0:65, :])
                    nc.sync.dma_start(dbg["dbg_at"], dtmp[:])
                # evacuate numerator + denominator (frees the psum bank in
                # ~0.7us); reciprocal chain trails off-PSUM, and the
                # normalize lags one head so the DVE never stalls on the
                # gpsimd broadcast
                den = sm_pool.tile([1, SH], F32, tag="den")
                nc.vector.tensor_copy(den[:], at_ps[64:65, :])
                nc.vector.tensor_copy(att_n[h][:], at_ps[0:D, :])
                rec = sm_pool.tile([1, SH], F32, tag="rec")
                nc.vector.reciprocal(rec[:], den[:])
                rb = sm_pool.tile([D, SH], F32, tag="rb")
                nc.gpsimd.partition_broadcast(rb[:], rec[:])
                if pending is not None:
                    ph, prb = pending
                    nc.vector.tensor_mul(
                        af[ph // 2][(ph % 2) * D:(ph % 2) * D + D, :],
                        att_n[ph][:], prb[:])
                pending = (h, rb)
        ph, prb = pending
        nc.vector.tensor_mul(af[ph // 2][(ph % 2) * D:(ph % 2) * D + D, :],
                             att_n[ph][:], prb[:])
        if dbg:
            nc.sync.dma_start(dbg["dbg_af"], af[0][:])

        # ---------------- output projection ----------------
        with tc.tile_pool(name="ps_pj", bufs=8, space="PSUM") as pj:
            pjt = [pj.tile([128, 384], F32, tag="pj", name=f"pj{i}")
                   for i in range(8)]
            for ft in range(6):
                wt = bwpool.tile([128, H], BF16, tag="wo")
                nc.sync.dma_start(wt[:], d_ow[ft * 128:(ft + 1) * 128, :])
                for nb in range(4):
                    for hf in range(2):
                        nc.tensor.matmul(
                            pjt[nb * 2 + hf][:],
                            af[ft][:, nb * 128:(nb + 1) * 128],
                            wt[:, hf * 384:(hf + 1) * 384],
                            start=(ft == 0), stop=(ft == 5))
            for nb in range(4):
                ot_sb = out_pool.tile([128, H], F32, tag="o")
                for hf in range(2):
                    nc.vector.tensor_add(ot_sb[:, hf * 384:(hf + 1) * 384],
                                         pjt[nb * 2 + hf][:],
                                         bias_bc[:, hf * 384:(hf + 1) * 384])
                nc.sync.dma_start(d_out[nb * 128:(nb + 1) * 128, :], ot_sb[:])

        attn_ctx.close()


def _get_program():
    if "nc" not in _CACHE:
        _CACHE["nc"] = _build_program()
    return _CACHE["nc"]


def _run(inputs, trace=False, **kw):
    nc = _get_program()
    in_maps = _host_prep(inputs)
    res = bass_utils.run_bass_kernel_spmd(
        nc, in_maps, core_ids=list(range(N_CORES)), trace=trace, **kw)
    full = np.empty((B, S, H), np.float32)
    for c in range(N_CORES):
        b, half = c // 2, c % 2
        full[b, half * SH:(half + 1) * SH, :] = res.results[c]["out"]
    return full, res


def kernel(**inputs):
    full, _ = _run(inputs)
    return full


# revision 71
# speedup vs baseline: 1.2297x; 1.2297x over previous
"""Fused KAN-linear multi-head-attention kernel for 8 Trainium2 NeuronCores.

Sharding: data-parallel over (batch=4) x (sequence halves=2) -> 8 cores.
Each core computes Q/K/V KAN projections for its 512 tokens, pairs exchange
rope'd K and token-major V via an in-pair AllGather, then each core runs
attention for its 512 queries over the full 1024-key sequence and the output
projection for its tokens.

The KAN spline bases are evaluated in closed form: on the uniform grid all 8
cubic B-spline bases are translates of the cardinal cubic B-spline,
  B_c(t) = (1/6)[u^3 - 4 relu(u-1)^3 + 6 relu(u-2)^3 - 4 relu(u-3)^3],
  u = clamp(t - c, 0, 4),  t = 2.5 x + 5.5,
computed from clamped cubes of relu(t-k) with exact zero tails.  The 1/6 and
the per-(out,in) scaler are folded into the spline weights host-side.

The spline contraction (6144 deep, 89% of the KAN FLOPs) runs in fp8e4m3
with DoubleRow perf mode: weights and bases are quantized to fp8 (weights
pre-scaled x512 to sit in e4m3's normal range), consecutive k-tile pairs are
packed into [128, 2, N] APs, and each matmul contracts 256 rows per pass.
The base (silu) matmul runs in bf16 with weights also pre-scaled x512 so
both accumulate into the same PSUM; the 1/512 is folded into the
PSUM-evacuation copy for free.  Q/K/V, the attention operands and the output
projection all run in bf16 (fp32r streams at half rate on the PE, bf16 at
full rate; this also halves the K/V pair-exchange).  The cube chain for the
spline bases is split across Scalar/Vector/GpSimd so basis production keeps
up with the fp8 matmul stream, and the softmax reciprocals are batched into
one 12-partition DVE op instead of twelve 1-partition ones.
"""

import numpy as np
import ml_dtypes

import concourse.bass as bass
import concourse.tile as tile
from concourse import bacc, mybir
from concourse import bass_utils

F32 = mybir.dt.float32
F32R = mybir.dt.float32r
BF16 = mybir.dt.bfloat16
FP8 = mybir.dt.float8e4
ALU = mybir.AluOpType
ACTF = mybir.ActivationFunctionType
DR = mybir.MatmulPerfMode.DoubleRow

B, S, H = 4, 1024, 768
HEADS, D = 12, 64
SH = 512                  # tokens per core
NDK = 24                  # spline double-k-tiles (48 k-tiles paired)
N_CORES = 8
ROPE_THETA = 10000.0
WSCALE = 512.0            # host pre-scale on all KAN weights
INV_WSCALE = 1.0 / WSCALE

K_ELEMS = H * SH                # rope'd K, feature-major [768, 512]
VT_COLS = HEADS * 65            # token-major V + per-head ones column
V_ELEMS = SH * VT_COLS          # [512, 780]
KV_ELEMS = K_ELEMS + V_ELEMS

_CACHE = {}
DEBUG = False


def _trunc_f32r(a):
    a = np.ascontiguousarray(a, np.float32)
    return (a.view(np.uint32) & np.uint32(0xFFFFFC00)).view(np.float32)


def _host_prep(inputs):
    """Rearrange weights and build per-core input maps."""
    f32 = np.float32
    bf16 = ml_dtypes.bfloat16
    fp8 = ml_dtypes.float8_e4m3
    x = np.asarray(inputs["x"], f32)

    shared = {}
    for lin in ("q", "k", "v"):
        sw = np.asarray(inputs[f"{lin}_spline_w"], f32)      # (o, i, c)
        sc = np.asarray(inputs[f"{lin}_scaler"], f32)
        ws = sw * sc[..., None] * np.float32(WSCALE / 6.0)
        # rows ordered (i_block, c, i_within); cols = o
        t2 = ws.transpose(1, 2, 0).reshape(6, 128, 8, H)      # (ib, di, c, o)
        w48 = t2.transpose(0, 2, 1, 3).reshape(48, 128, H)    # kt=(ib*8+c)
        if lin == "v":
            # V error passes linearly to the output -> bf16 spline weights
            # (moving operand of the token-major matmul, fp8 bases stay
            # the stationary operand)
            shared["w2v"] = w48.reshape(48 * 128, H).astype(bf16)
        else:
            # DoubleRow pairs: [dk, di, sub, o] flattened to [3072, 1536]
            wdr = (w48.reshape(NDK, 2, 128, H).transpose(0, 2, 1, 3)
                   .reshape(NDK * 128, 2 * H))
            shared[f"w2{lin}"] = np.clip(wdr, -240.0, 240.0).astype(fp8)
        shared[f"bw{lin}"] = (
            np.asarray(inputs[f"{lin}_base_w"], f32).T * np.float32(WSCALE)
        ).astype(bf16)                                        # (i, o)
    shared["owT"] = np.asarray(inputs["out_w"], f32).T.astype(bf16)
    shared["ob"] = _trunc_f32r(
        np.asarray(inputs["out_b"], f32).reshape(1, H))

    # rotate-half matrix as lhsT: (lhsT.T @ u) = rot_half(u), 2 heads/tile
    rotT = np.zeros((128, 128), f32)
    for blk in (0, 64):
        for j in range(32):
            rotT[blk + j + 32, blk + j] = -1.0
            rotT[blk + j, blk + j + 32] = 1.0
    shared["rotT"] = _trunc_f32r(rotT)

    # rope tables per sequence half: [64 d, 512 pos] tiled to 128 partitions
    inv_freq = (1.0 / (ROPE_THETA ** (np.arange(0, D, 2, dtype=f32) / D))).astype(f32)
    cos_h, sin_h = [], []
    for half in range(2):
        pos = (half * SH + np.arange(SH)).astype(f32)
        freqs = np.outer(pos, inv_freq).astype(f32)           # (512, 32)
        emb = np.concatenate([freqs, freqs], axis=1)          # (512, 64)
        cos_h.append(np.tile(np.cos(emb).astype(f32).T, (2, 1)))  # (128, 512)
        sin_h.append(np.tile(np.sin(emb).astype(f32).T, (2, 1)))

    in_maps = []
    for c in range(N_CORES):
        b, half = c // 2, c % 2
        m = dict(shared)
        xT = x[b, half * SH:(half + 1) * SH, :].T          # (768, 512)
        # super-blocks: row j*128+p holds features (2j)*128+p | (2j+1)*128+p
        # side by side so the basis pipeline runs on [128, 1024] tiles
        m["xT"] = np.ascontiguousarray(
            xT.reshape(3, 2, 128, SH).transpose(0, 2, 1, 3).reshape(384, 2 * SH))
        m["cosT"] = cos_h[half]
        m["sinT"] = sin_h[half]
        in_maps.append(m)
    return in_maps


def _build_program():
    nc = bacc.Bacc("TRN2", target_bir_lowering=False, debug=False,
                   num_devices=N_CORES)

    d_xT = nc.dram_tensor("xT", [384, 2 * SH], F32, kind="ExternalInput").ap()
    d_cos = nc.dram_tensor("cosT", [128, SH], F32, kind="ExternalInput").ap()
    d_sin = nc.dram_tensor("sinT", [128, SH], F32, kind="ExternalInput").ap()
    d_rot = nc.dram_tensor("rotT", [128, 128], F32R, kind="ExternalInput").ap()
    d_w2 = {lin: nc.dram_tensor(f"w2{lin}", [NDK * 128, 2 * H], FP8,
                                kind="ExternalInput").ap()
            for lin in ("q", "k")}
    d_w2["v"] = nc.dram_tensor("w2v", [48 * 128, H], BF16,
                               kind="ExternalInput").ap()
    d_bw = {lin: nc.dram_tensor(f"bw{lin}", [H, H], BF16,
                                kind="ExternalInput").ap()
            for lin in ("q", "k", "v")}
    d_ow = nc.dram_tensor("owT", [H, H], BF16, kind="ExternalInput").ap()
    d_ob = nc.dram_tensor("ob", [1, H], F32R, kind="ExternalInput").ap()
    d_out = nc.dram_tensor("out", [SH, H], F32, kind="ExternalOutput").ap()
    dbg = {}
    if DEBUG:
        for nm, shp, dt in [("dbg_bases", [128, 4 * SH], FP8),
                            ("dbg_silu", [128, 2 * SH], BF16),
                            ("dbg_kraw", [128, SH], F32),
                            ("dbg_kf", [128, SH], BF16),
                            ("dbg_vt", [128, VT_COLS], BF16),
                            ("dbg_qf", [128, SH], BF16),
                            ("dbg_ka", [128, SH], BF16),
                            ("dbg_va", [128, VT_COLS], BF16),
                            ("dbg_pt", [128, SH], BF16),
                            ("dbg_at", [128, SH], F32),
                            ("dbg_af", [128, SH], BF16)]:
            dbg[nm] = nc.dram_tensor(nm, shp, dt, kind="ExternalOutput").ap()

    with tile.TileContext(nc) as tc:
        _emit(nc, tc, d_xT, d_cos, d_sin, d_rot, d_w2, d_bw, d_ow, d_ob, d_out,
              dbg)
    nc.compile()
    return nc


def _emit(nc, tc, d_xT, d_cos, d_sin, d_rot, d_w2, d_bw, d_ow, d_ob, d_out,
          dbg={}):
    from contextlib import ExitStack

    ctx = ExitStack()
    with ctx:
        const = ctx.enter_context(tc.tile_pool(name="const", bufs=1))
        wpool = ctx.enter_context(tc.tile_pool(name="wpool", bufs=4))
        bwpool = ctx.enter_context(tc.tile_pool(name="bwpool", bufs=2))
        qf_pool = ctx.enter_context(tc.tile_pool(name="qfp", bufs=6))
        kf_pool = ctx.enter_context(tc.tile_pool(name="kfp", bufs=2))
        vt_pool = ctx.enter_context(tc.tile_pool(name="vtp", bufs=2))
        dram = ctx.enter_context(tc.tile_pool(name="dram", bufs=1, space="DRAM"))

        cosT = const.tile([128, SH], F32, tag="cos")
        nc.sync.dma_start(cosT[:], d_cos[:])
        sinT = const.tile([128, SH], F32, tag="sin")
        nc.sync.dma_start(sinT[:], d_sin[:])
        rotT = const.tile([128, 128], F32R, tag="rot")
        nc.sync.dma_start(rotT[:], d_rot[:])
        ob_sb = const.tile([1, H], F32, tag="ob")
        nc.sync.dma_start(ob_sb[:], d_ob.bitcast(F32)[:])
        bias_bc = const.tile([128, H], F32, tag="biasbc")
        nc.gpsimd.partition_broadcast(bias_bc[:], ob_sb[:])

        kvk_in = dram.tile([K_ELEMS], BF16, tag="kvkin")
        kvk_out = dram.tile([2 * K_ELEMS], BF16, tag="kvkout")
        kvv_in = dram.tile([V_ELEMS], BF16, tag="kvvin")
        kvv_out = dram.tile([2 * V_ELEMS], BF16, tag="kvvout")

        neg_k = []
        for k in range(11):
            bk = const.tile([128, 1], F32, tag=f"bk{k}")
            nc.vector.memset(bk[:], float(-k))
            neg_k.append(bk)
        ones_p = const.tile([128, 1], F32, tag="onesp")
        nc.vector.memset(ones_p[:], 1.0)
        rbias = {}
        for v in (64.0, 108.0, 48.0, 4.0):
            bv = const.tile([128, 1], F32, tag=f"rb{int(v)}")
            nc.vector.memset(bv[:], v)
            rbias[v] = bv

        silu = []
        bases = {}                # (ib, m) -> [128, 2*SH] fp8 pair tile

        # ---------------- bases + silu ----------------
        bases_ctx = ExitStack()
        bases_pool = bases_ctx.enter_context(
            tc.tile_pool(name="basesp", bufs=12))
        silu_pool = bases_ctx.enter_context(tc.tile_pool(name="silup", bufs=3))
        rs_pool = bases_ctx.enter_context(tc.tile_pool(name="rsp", bufs=2))
        scr = bases_ctx.enter_context(tc.tile_pool(name="scr", bufs=2))
        scr_kp = bases_ctx.enter_context(tc.tile_pool(name="scrkp", bufs=5))
        scr_b = bases_ctx.enter_context(tc.tile_pool(name="scrb", bufs=5))
        xp = bases_ctx.enter_context(tc.tile_pool(name="xp", bufs=2))

        # One pass per super-block j covers real feature blocks (2j, 2j+1)
        # side by side in [128, 1024] tiles: halves the elementwise op count
        # so the per-op engine overhead is paid half as often.
        W = 2 * SH
        for j in range(3):
            xt = xp.tile([128, W], F32, tag="x")
            nc.sync.dma_start(xt[:], d_xT[j * 128:(j + 1) * 128, :])
            sl = silu_pool.tile([128, W], BF16, tag="silu")
            nc.scalar.activation(sl[:], xt[:], ACTF.Silu)
            silu.append(sl)

            tt = scr.tile([128, W], F32, tag="t")
            nc.vector.tensor_scalar(tt[:], xt[:], 2.5, 5.5, ALU.mult, ALU.add)

            # chain intermediates in bf16. DVE is ALU-bound (not stream-
            # bound), so minimize op cost: plain cubes via tensor_tensor,
            # each clamped term as ONE tensor_scalar (scale folded into the
            # clamp op), and plain tensor_tensor adds to combine:
            #   B~ = min(C,64) + max(-4C',-108) + min(6C'',48) + max(-4C''',-4)
            M0 = {}
            M1 = {}
            M2 = {}
            M3 = {}
            CB = {}
            for k in range(11):
                R = scr.tile([128, W], BF16, tag="R")
                nc.scalar.activation(R[:], tt[:], ACTF.Relu, bias=neg_k[k][:])
                Sq = scr.tile([128, W], BF16, tag="Sq")
                nc.scalar.activation(Sq[:], R[:], ACTF.Square)
                cb = scr_kp.tile([128, W], BF16, tag="Kp")
                nc.vector.tensor_mul(cb[:], Sq[:], R[:])   # relu(t-k)^3
                CB[k] = cb
                if k <= 5:
                    m0 = scr_b.tile([128, W], BF16, tag="m0")
                    nc.vector.tensor_scalar(m0[:], cb[:], 1.0, 64.0,
                                            ALU.mult, ALU.min)
                    M0[k] = m0
                if 1 <= k <= 6:
                    m1 = scr_b.tile([128, W], BF16, tag="m1")
                    nc.vector.tensor_scalar(m1[:], cb[:], -4.0, -108.0,
                                            ALU.mult, ALU.max)
                    M1[k] = m1
                if 2 <= k <= 7:
                    m2 = scr_b.tile([128, W], BF16, tag="m2")
                    nc.vector.tensor_scalar(m2[:], cb[:], 6.0, 48.0,
                                            ALU.mult, ALU.min)
                    M2[k] = m2
                if 3 <= k <= 8:
                    m3 = scr_b.tile([128, W], BF16, tag="m3")
                    nc.vector.tensor_scalar(m3[:], cb[:], -4.0, -4.0,
                                            ALU.mult, ALU.max)
                    M3[k] = m3
                if k >= 3:
                    c = k - 3
                    if c % 2 == 0:
                        bp = bases_pool.tile([128, 2 * W], FP8, tag="bases")
                        bases[(j, c // 2)] = bp
                    bp4 = bases[(j, c // 2)].rearrange(
                        "p (i s n) -> p i s n", i=2, s=2)
                    acc = scr.tile([128, W], BF16, tag="acc")
                    if c < 6:
                        nc.vector.tensor_add(acc[:], M0[c][:], M1[c + 1][:])
                        nc.vector.tensor_add(acc[:], acc[:], M2[c + 2][:])
                        # final add -> fp8 quarter of the 4D tile
                        # layout [128, (ib2, s, n)]: per real ib a
                        # [128, 2, 512] DoubleRow pair AP, ib2 axis strided.
                        nc.vector.tensor_add(
                            bp4[:, :, c % 2, :],
                            acc.rearrange("p (i n) -> p i n", i=2),
                            M3[c + 3].rearrange("p (i n) -> p i n", i=2))
                    else:
                        # ACT-relu form for the last two bases rebalances the
                        # clamp work from the (saturated) DVE onto Scalar:
                        # B~ = (r1 + r3) - (r0 + r2), r_j = relu(b_j - a_j*C)
                        # with the shift constants cancelling exactly.
                        r0 = scr_b.tile([128, W], BF16, tag="m0")
                        nc.scalar.activation(r0[:], CB[c][:], ACTF.Relu,
                                             scale=-1.0, bias=rbias[64.0][:])
                        r1 = scr_b.tile([128, W], BF16, tag="m1")
                        nc.scalar.activation(r1[:], CB[c + 1][:], ACTF.Relu,
                                             scale=-4.0, bias=rbias[108.0][:])
                        r2 = scr_b.tile([128, W], BF16, tag="m2")
                        nc.scalar.activation(r2[:], CB[c + 2][:], ACTF.Relu,
                                             scale=-6.0, bias=rbias[48.0][:])
                        r3 = scr_b.tile([128, W], BF16, tag="m3")
                        nc.scalar.activation(r3[:], CB[c + 3][:], ACTF.Relu,
                                             scale=-4.0, bias=rbias[4.0][:])
                        nc.vector.tensor_add(acc[:], r1[:], r3[:])
                        a2 = scr.tile([128, W], BF16, tag="acc2")
                        nc.vector.tensor_add(a2[:], r0[:], r2[:])
                        nc.vector.tensor_sub(
                            bp4[:, :, c % 2, :],
                            acc.rearrange("p (i n) -> p i n", i=2),
                            a2.rearrange("p (i n) -> p i n", i=2))
            del M0, M1, M2, M3, CB
            if dbg and j == 0:
                nc.sync.dma_start(dbg["dbg_bases"], bases[(0, 3)][:])
                nc.sync.dma_start(dbg["dbg_silu"], silu[0][:])

        # ---------------- KAN matmul phases ----------------
        def spline_w(lin, dk):
            wt = wpool.tile([128, 2 * H], FP8, tag="w8")
            nc.sync.dma_start(wt[:], d_w2[lin][dk * 128:(dk + 1) * 128, :])
            return wt.rearrange("p (s o) -> p s o", s=2)

        def base_w(lin, ib):
            bwt = bwpool.tile([128, H], BF16, tag="wb")
            nc.sync.dma_start(bwt[:], d_bw[lin][ib * 128:(ib + 1) * 128, :])
            return bwt

        def feature_major_phase(lin, out_pool, out_tag, bounce_base):
            """Q/K: out[o, n] with rope; optionally DMA to exchange buffer."""
            outs = []
            with tc.tile_pool(name=f"ps_{lin}", bufs=6, space="PSUM") as psp, \
                 tc.tile_pool(name=f"aux_{lin}", bufs=2, space="PSUM") as aux:
                ps = [psp.tile([128, SH], F32, tag="kan", name=f"kan{i}")
                      for i in range(6)]
                # base (silu) matmuls first: they only need silu, so the PE
                # has work while the spline bases are still being produced
                for ib in range(6):
                    bwt = base_w(lin, ib)
                    sl = silu[ib // 2][:, (ib % 2) * SH:(ib % 2 + 1) * SH]
                    for ot in range(6):
                        nc.tensor.matmul(ps[ot][:],
                                         bwt[:, ot * 128:(ot + 1) * 128],
                                         sl,
                                         start=(ib == 0), stop=False)
                for dk in range(NDK):
                    ib, m = dk // 4, dk % 4
                    w3 = spline_w(lin, dk)
                    b3 = bases[(ib // 2, m)].rearrange(
                        "p (i s n) -> p i s n", i=2, s=2)[:, ib % 2, :, :]
                    for ot in range(6):
                        nc.tensor.matmul(ps[ot][:],
                                         w3[:, :, ot * 128:(ot + 1) * 128],
                                         b3[:, :, :],
                                         start=False, stop=(dk == NDK - 1),
                                         perf_mode=DR)
                for ot in range(6):
                    raw = rs_pool.tile([128, SH], F32R, tag="raw")
                    nc.scalar.activation(raw[:], ps[ot][:], ACTF.Copy,
                                         scale=INV_WSCALE)
                    if dbg and lin == "k" and ot == 0:
                        nc.sync.dma_start(dbg["dbg_kraw"], raw.bitcast(F32)[:])
                    rp = aux.tile([128, SH], F32, tag="rope")
                    nc.tensor.matmul(rp[:], rotT[:], raw[:],
                                     start=True, stop=True)
                    t1 = rs_pool.tile([128, SH], F32, tag="rt1")
                    nc.vector.tensor_mul(t1[:], raw.bitcast(F32)[:], cosT[:])
                    t2 = rs_pool.tile([128, SH], F32, tag="rt2")
                    nc.vector.tensor_mul(t2[:], rp[:], sinT[:])
                    out = out_pool.tile([128, SH], BF16, tag=out_tag)
                    nc.vector.tensor_add(out[:], t1[:], t2[:])
                    outs.append(out)
                    if dbg and ot == 0:
                        nc.sync.dma_start(
                            dbg["dbg_kf" if lin == "k" else "dbg_qf"], out[:])
                    if bounce_base is not None:
                        off = bounce_base + ot * 128 * SH
                        dst = kvk_in[off:off + 128 * SH].rearrange(
                            "(p f) -> p f", f=SH)
                        nc.sync.dma_start(dst, out[:])
            return outs

        def token_major_v_phase():
            """V: out[n, heads*65] with ones columns, to exchange buffer."""
            with tc.tile_pool(name="ps_v", bufs=8, space="PSUM") as pv:
                ps = [pv.tile([128, 384], F32, tag="vps", name=f"vps{i}")
                      for i in range(8)]
                for ib in range(6):
                    bwt = base_w("v", ib)
                    sl = silu[ib // 2]
                    for nb in range(4):
                        for hf in range(2):
                            nc.tensor.matmul(
                                ps[nb * 2 + hf][:],
                                sl[:, (ib % 2) * SH + nb * 128:
                                   (ib % 2) * SH + (nb + 1) * 128],
                                bwt[:, hf * 384:(hf + 1) * 384],
                                start=(ib == 0), stop=False)
                for kt in range(48):
                    ib, c = kt // 8, kt % 8
                    wt = wpool.tile([128, H], BF16, tag="wv")
                    nc.sync.dma_start(wt[:],
                                      d_w2["v"][kt * 128:(kt + 1) * 128, :])
                    bp4 = bases[(ib // 2, c // 2)].rearrange(
                        "p (i s n) -> p i s n", i=2, s=2)
                    for nb in range(4):
                        for hf in range(2):
                            nc.tensor.matmul(
                                ps[nb * 2 + hf][:],
                                bp4[:, ib % 2, c % 2,
                                    nb * 128:(nb + 1) * 128],
                                wt[:, hf * 384:(hf + 1) * 384],
                                start=False, stop=(kt == 47))
                for nb in range(4):
                    vt = vt_pool.tile([128, VT_COLS], BF16, tag="vt")
                    vt3 = vt.rearrange("p (h e) -> p h e", e=65)
                    nc.vector.tensor_copy(
                        vt3[:, :, 64:65],
                        ones_p[:].unsqueeze(2).to_broadcast([128, HEADS, 1]))
                    for hf in range(2):
                        src = ps[nb * 2 + hf].rearrange("p (h d) -> p h d", d=D)
                        nc.vector.tensor_scalar_mul(
                            vt3[:, hf * 6:(hf + 1) * 6, 0:D], src[:],
                            INV_WSCALE)
                    off = nb * 128 * VT_COLS
                    dst = kvv_in[off:off + 128 * VT_COLS].rearrange(
                        "(p f) -> p f", f=VT_COLS)
                    nc.sync.dma_start(dst, vt[:])
                    if dbg and nb == 0:
                        nc.sync.dma_start(dbg["dbg_vt"], vt[:])

        # V first: its per-basis matmul volume best matches the basis
        # production rate, so the PE stays dense while bases trickle in.
        # Each collective starts as soon as its operand is exchanged-ready
        # and hides under the next compute phase.
        groups = [[i, i + 1] for i in range(0, N_CORES, 2)]
        token_major_v_phase()
        nc.gpsimd.collective_compute(
            "AllGather", ALU.bypass, replica_groups=groups,
            ins=[kvv_in.opt()], outs=[kvv_out.opt()])
        feature_major_phase("k", kf_pool, "kf", bounce_base=0)
        nc.gpsimd.collective_compute(
            "AllGather", ALU.bypass, replica_groups=groups,
            ins=[kvk_in.opt()], outs=[kvk_out.opt()])
        qf = feature_major_phase("q", qf_pool, "qf", bounce_base=None)

        # close bases/scratch pools before attention working set opens
        bases_ctx.close()

        # ---------------- attention ----------------
        attn_ctx = ExitStack()
        ka_pool = attn_ctx.enter_context(tc.tile_pool(name="kap", bufs=12))
        va_pool = attn_ctx.enter_context(tc.tile_pool(name="vap", bufs=8))
        pb_pool = attn_ctx.enter_context(tc.tile_pool(name="pbp", bufs=10))
        an_pool = attn_ctx.enter_context(tc.tile_pool(name="anp", bufs=12))
        af_pool = attn_ctx.enter_context(tc.tile_pool(name="afp", bufs=6))
        sm_pool = attn_ctx.enter_context(tc.tile_pool(name="smp", bufs=4))
        out_pool = attn_ctx.enter_context(tc.tile_pool(name="outp", bufs=4))

        k_all = []
        for hf in range(2):
            for ot in range(6):
                t = ka_pool.tile([128, SH], BF16, tag="ka")
                off = hf * K_ELEMS + ot * 128 * SH
                nc.sync.dma_start(
                    t[:],
                    kvk_out[off:off + 128 * SH].rearrange("(p f) -> p f", f=SH))
                if dbg and hf == 1 and ot == 0:
                    nc.sync.dma_start(dbg["dbg_ka"], t[:])
                k_all.append(t)
        v_all = []
        for hf in range(2):
            for nb in range(4):
                t = va_pool.tile([128, VT_COLS], BF16, tag="va")
                off = hf * V_ELEMS + nb * 128 * VT_COLS
                nc.sync.dma_start(
                    t[:],
                    kvv_out[off:off + 128 * VT_COLS].rearrange(
                        "(p f) -> p f", f=VT_COLS))
                if dbg and hf == 1 and nb == 0:
                    nc.sync.dma_start(dbg["dbg_va"], t[:])
                v_all.append(t)

        af = [af_pool.tile([128, SH], BF16, tag="af", name=f"af{i}")
              for i in range(6)]

        # numerator + denominator evacuate to SBUF right after each head's
        # accumulation, so the at_ps bank frees in ~0.7us and the reciprocal
        # chain (reciprocal -> broadcast -> normalize) trails off-PSUM.
        # Everything stays at partition base 0: non-zero partition bases in
        # the copy/broadcast path silently misbehave on hardware.
        att_n = [an_pool.tile([D, SH], F32, tag="attn", name=f"attn{h}")
                 for h in range(HEADS)]

        pending = None
        with tc.tile_pool(name="ps_sc", bufs=3, space="PSUM") as scp, \
             tc.tile_pool(name="ps_at", bufs=2, space="PSUM") as atp:
            for h in range(HEADS):
                tf, r0 = h // 2, (h % 2) * D
                at_ps = atp.tile([128, SH], F32, tag="at")
                pts = []
                # score matmuls fill a 2-bank psum pair-wise; one exp call
                # covers both banks; the attn@V accumulations follow so exp
                # latency hides under the score stream
                for pr in range(4):
                    sc_ps = scp.tile([128, 2 * SH], F32, tag="sc")
                    for i in range(2):
                        kb = pr * 2 + i
                        hf, blk = kb // 4, kb % 4
                        nc.tensor.matmul(
                            sc_ps[:, i * SH:(i + 1) * SH],
                            k_all[hf * 6 + tf][r0:r0 + D,
                                               blk * 128:(blk + 1) * 128],
                            qf[tf][r0:r0 + D, :],
                            start=True, stop=True)
                    pt = pb_pool.tile([128, 2 * SH], BF16, tag="pt")
                    nc.scalar.activation(pt[:], sc_ps[:], ACTF.Exp,
                                         scale=0.125)
                    if dbg and h == 0 and pr == 0:
                        nc.sync.dma_start(dbg["dbg_pt"], pt[:, 0:SH])
                    pts.append(pt)
                for kb in range(8):
                    nc.tensor.matmul(at_ps[0:65, :],
                                     v_all[kb][:, h * 65:h * 65 + 65],
                                     pts[kb // 2][:, (kb % 2) * SH:
                                                  (kb % 2 + 1) * SH],
                                     start=(kb == 0), stop=(kb == 7))
                if dbg and h == 0:
                    dtmp = sm_pool.tile([128, SH], F32, tag="dbgat")
                    nc.vector.memset(dtmp[:], 0.0)
                    nc.vector.tensor_copy(dtmp[0:65, :], at_ps[0:65, :])
                    nc.sync.dma_start(dbg["dbg_at"], dtmp[:])
                # evacuate numerator + denominator (frees the psum bank in
                # ~0.7us); reciprocal chain trails off-PSUM, and the
                # normalize lags one head so the DVE never stalls on the
                # gpsimd broadcast
                den = sm_pool.tile([1, SH], F32, tag="den")
                nc.vector.tensor_copy(den[:], at_ps[64:65, :])
                nc.vector.tensor_copy(att_n[h][:], at_ps[0:D, :])
                rec = sm_pool.tile([1, SH], F32, tag="rec")
                nc.vector.reciprocal(rec[:], den[:])
                rb = sm_pool.tile([D, SH], F32, tag="rb")
                nc.gpsimd.partition_broadcast(rb[:], rec[:])
                if pending is not None:
                    ph, prb = pending
                    nc.vector.tensor_mul(
                        af[ph // 2][(ph % 2) * D:(ph % 2) * D + D, :],
                        att_n[ph][:], prb[:])
                pending = (h, rb)
        ph, prb = pending
        nc.vector.tensor_mul(af[ph // 2][(ph % 2) * D:(ph % 2) * D + D, :],
                             att_n[ph][:], prb[:])
        if dbg:
            nc.sync.dma_start(dbg["dbg_af"], af[0][:])

        # ---------------- output projection ----------------
        with tc.tile_pool(name="ps_pj", bufs=8, space="PSUM") as pj:
            pjt = [pj.tile([128, 384], F32, tag="pj", name=f"pj{i}")
                   for i in range(8)]
            for ft in range(6):
                wt = bwpool.tile([128, H], BF16, tag="wo")
                nc.sync.dma_start(wt[:], d_ow[ft * 128:(ft + 1) * 128, :])
                for nb in range(4):
                    for hf in range(2):
                        nc.tensor.matmul(
                            pjt[nb * 2 + hf][:],
                            af[ft][:, nb * 128:(nb + 1) * 128],
                            wt[:, hf * 384:(hf + 1) * 384],
                            start=(ft == 0), stop=(ft == 5))
            for nb in range(4):
                ot_sb = out_pool.tile([128, H], F32, tag="o")
                for hf in range(2):
                    nc.vector.tensor_add(ot_sb[:, hf * 384:(hf + 1) * 384],
                                         pjt[nb * 2 + hf][:],
                                         bias_bc[:, hf * 384:(hf + 1) * 384])
                nc.sync.dma_start(d_out[nb * 128:(nb + 1) * 128, :], ot_sb[:])

        attn_ctx.close()


def _get_program():
    if "nc" not in _CACHE:
        _CACHE["nc"] = _build_program()
    return _CACHE["nc"]


def _run(inputs, trace=False, **kw):
    nc = _get_program()
    in_maps = _host_prep(inputs)
    res = bass_utils.run_bass_kernel_spmd(
        nc, in_maps, core_ids=list(range(N_CORES)), trace=trace, **kw)
    full = np.empty((B, S, H), np.float32)
    for c in range(N_CORES):
        b, half = c // 2, c % 2
        full[b, half * SH:(half + 1) * SH, :] = res.results[c]["out"]
    return full, res


def kernel(**inputs):
    full, _ = _run(inputs)
    return full
